# revision 26
# baseline (speedup 1.0000x reference)
"""Bass/Tile kernel for masked dot-product attention on 8 Trainium2 cores.

Problem: queries/keys/values [128, 1024, 64] fp32, valid_lens [128] int32.
  out[b] = softmax(mask(Q K^T / 8, valid_lens[b])) @ V

Strategy (v2):
  * Shard the 128 batch*heads across 8 cores, 16 head-slots per core.
    Heads are sorted by valid_len (descending) and dealt in consecutive
    groups of 8, one head per core per slot -> one SPMD program whose
    per-slot key-chunk count m = ceil(max valid_len in group / 128).
  * All layout work happens on the HOST: Q^T / K^T fp16 panels and
    [V_chunk | ones] fp16 blocks are built with numpy, so the device does
    zero transposes, zero casts of inputs, zero memsets per head.
  * Device pipeline per key chunk c (the ScalarE exp is the pacer):
      S^T = K_c Q^T            2 matmuls  (PE, K=64, N=512, fp16)
      P^T = exp(S^T/8 + mask)  1 activation [128,1024] (ACT, bias = mask col)
      [O^T ; denom] += [V_c|1]^T P^T   2 matmuls (PE, M=65, N=512)
    The masked key positions get bias -1e6 -> exp underflows to exactly 0,
    so fully-masked chunks beyond valid_len are simply skipped and no max
    subtraction is needed (scores are bounded, fp32 reference-exact).
  * The denominator rides along as a 65th stationary column of ones, so
    one PSUM accumulator holds [O^T ; sum_k P^T].  The device just casts
    it to fp16 and ships it; normalization (num/den) and the final
    transpose back to [q, d] happen on the host in numpy.
  * Heads with valid_len == 0 (reference: uniform attention) are fixed up
    on the host with the exact reference semantics (mean of V).
"""

import math
from contextlib import ExitStack

import numpy as np

import concourse.bass as bass  # noqa: F401  (engine namespaces live on the nc)
import concourse.mybir as mybir
import concourse.tile as tile
from concourse import bacc
from concourse.bass_utils import run_bass_kernel_spmd

BH, L, D = 128, 1024, 64
NCORES = 8
SLOTS = BH // NCORES  # 16
CHUNK = 128
NCH = L // CHUNK  # 8
MASK_VALUE = -1000000.0
F32 = mybir.dt.float32
MM_DT = mybir.dt.float16  # 1 cyc/row on PE, ~2^-11 operand quantization

NPAIR = NCH // 2  # key chunks are processed in row-tiled pairs
QKW = L + NPAIR * CHUNK  # qk panel max width: Q^T (1024) + K^T pair blocks
VPW = NCH * (D + 1)  # vp panel max width: [V_c | ones] blocks of 65

_program_cache: dict = {}


def _build_program(m_list):
    nc = bacc.Bacc("TRN2", target_bir_lowering=False, debug=False)
    qk_d = nc.dram_tensor("qk", [SLOTS, 128, QKW], MM_DT, kind="ExternalInput").ap()
    vp_d = nc.dram_tensor("vp", [SLOTS, 128, VPW], MM_DT, kind="ExternalInput").ap()
    mb_d = nc.dram_tensor("mb", [CHUNK, SLOTS * NCH], F32, kind="ExternalInput").ap()
    ot_d = nc.dram_tensor("ot", [SLOTS, D + 1, L], MM_DT, kind="ExternalOutput").ap()

    Exp = mybir.ActivationFunctionType.Exp

    with tile.TileContext(nc) as tc, ExitStack() as ctx:
        const = ctx.enter_context(tc.tile_pool(name="const", bufs=1))
        mb = const.tile([CHUNK, SLOTS * NCH], F32)
        nc.sync.dma_start(mb[:], mb_d[:])
        ones = const.tile([128, 1], F32)
        nc.vector.memset(ones[:], 1.0)
        actwarm = const.tile([128, 1], F32, tag="actwarm")
        nc.scalar.activation(actwarm[:], ones[:], Exp, bias=0.0, scale=1.0)

        qk_p = ctx.enter_context(tc.tile_pool(name="qk", bufs=3))
        vp_p = ctx.enter_context(tc.tile_pool(name="vp", bufs=3))
        pts_p = ctx.enter_context(tc.tile_pool(name="pts", bufs=5))
        osb_p = ctx.enter_context(tc.tile_pool(name="osb", bufs=3))

        # PSUM: 8 banks. s: S^T tiles (2 x 2 banks); ot: [O^T;den]
        # accumulators, one bank per q-half, double-buffered across heads.
        s_ps = ctx.enter_context(tc.tile_pool(name="s", bufs=3, space="PSUM"))
        o_ps = ctx.enter_context(tc.tile_pool(name="ot", bufs=2, space="PSUM"))

        def load_head(j):
            g = (m_list[j] + 1) // 2  # pair blocks
            qk = qk_p.tile([128, QKW], MM_DT, tag="qk", name=f"qk{j}")
            nc.sync.dma_start(
                qk[:, 0 : L + g * CHUNK], qk_d[j, :, 0 : L + g * CHUNK]
            )
            vp = vp_p.tile([128, VPW], MM_DT, tag="vp", name=f"vp{j}")
            nc.gpsimd.dma_start(vp[:, 0 : m_list[j] * 65], vp_d[j, :, 0 : m_list[j] * 65])
            return qk, vp

        # Dense matmul burst to flip the PE HAM clock-gate to full rate
        # (needs ~3.4us of contiguous PE activity; at the cold 1.2 GHz rate
        # 6 x N=512 matmuls ~= 3.8us) while the first panel DMAs are in
        # flight.  Too short a burst leaves the whole kernel at K=4/8.
        # The memset goes first on the otherwise-idle gpsimd queue so the
        # burst can start right after the framework preamble.
        warm = const.tile([128, 512], MM_DT, tag="warm")
        nc.gpsimd.memset(warm[:], 0.5)

        heads = {j: None for j in range(SLOTS)}
        for j in range(min(3, SLOTS)):
            heads[j] = load_head(j)

        wps = o_ps.tile([128, 512], F32, tag="ot")  # noqa
        for i in range(6):
            nc.tensor.matmul(wps[:], warm[:, 0:128], warm[:], start=True, stop=True)

        # Key chunks are processed in pairs: the even chunk's K^T lives in
        # SBUF partitions 0:64, the odd chunk's in 64:128 (Q^T is duplicated
        # in both halves).  The paired S^T matmuls go to different PE row
        # groups (tile_position (0,0) / (64,0)) and run concurrently.
        pairs = []
        for j in range(SLOTS):
            for g in range((m_list[j] + 1) // 2):
                cs = [2 * g] + ([2 * g + 1] if 2 * g + 1 < m_list[j] else [])
                pairs.append((j, g, cs))
        state: dict = {}  # (j, c) -> pts tile
        otiles: dict = {}  # j -> [ot_h0, ot_h1]

        def emit_pv(j, c):
            qk, vp = heads[j]
            m = m_list[j]
            if c == 0:
                otiles[j] = [
                    o_ps.tile([D + 1, 512], F32, tag="ot", name=f"ot{j}_{h}")
                    for h in range(2)
                ]
            pts = state.pop((j, c))
            for h in range(2):
                nc.tensor.matmul(
                    otiles[j][h][:],
                    vp[:, c * 65 : (c + 1) * 65],
                    pts[:, h * 512 : (h + 1) * 512],
                    start=(c == 0),
                    stop=(c == m - 1),
                )
            if c == m - 1:
                # [O^T ; denom] -> fp16 -> DRAM; host divides + transposes.
                osb = osb_p.tile([D + 1, L], MM_DT, tag="osb", name=f"osb{j}")
                for h in range(2):
                    nc.vector.tensor_copy(
                        osb[:, h * 512 : (h + 1) * 512], otiles[j][h][:]
                    )
                    nc.sync.dma_start(
                        ot_d[j, :, h * 512 : (h + 1) * 512],
                        osb[:, h * 512 : (h + 1) * 512],
                    )
                del otiles[j]

        prev_pair = None
        cur_head = -1
        for j, g, cs in pairs:
            if j != cur_head:
                cur_head = j
                if j + 3 < SLOTS:
                    heads[j + 3] = load_head(j + 3)
            qk, _ = heads[j]
            stiles = {
                c: s_ps.tile([128, L], F32, tag="s", name=f"s{j}_{c}") for c in cs
            }
            for h in range(2):
                for c in cs:
                    half = (c % 2) * 64
                    nc.tensor.matmul(
                        stiles[c][:, h * 512 : (h + 1) * 512],
                        qk[half : half + 64, L + g * CHUNK : L + (g + 1) * CHUNK],
                        qk[half : half + 64, h * 512 : (h + 1) * 512],
                        start=True,
                        stop=True,
                    )
            for c in cs:
                pts = pts_p.tile([128, L], MM_DT, tag="pts", name=f"pts{j}_{c}")
                col = j * NCH + c
                nc.scalar.activation(
                    pts[:], stiles[c][:], Exp, bias=mb[:, col : col + 1], scale=0.125
                )
                state[(j, c)] = pts
            if prev_pair is not None:
                for pj, pc in prev_pair:
                    emit_pv(pj, pc)
            prev_pair = [(j, c) for c in cs]
        for pj, pc in prev_pair:
            emit_pv(pj, pc)

    nc.compile()
    return nc


def _plan(valid_lens):
    """Sort heads by valid_len desc, deal consecutive groups of 8 across cores.

    Returns (assign [NCORES, SLOTS] head indices, m_list [SLOTS] chunk counts).
    """
    order = np.argsort(-valid_lens, kind="stable")
    assign = order.reshape(SLOTS, NCORES).T  # [core, slot]
    m_list = []
    for j in range(SLOTS):
        vmax = int(valid_lens[assign[:, j]].max())
        m_list.append(min(NCH, max(1, math.ceil(vmax / CHUNK))))
    return assign, m_list


def _run(queries, keys, values, valid_lens, trace=False):
    queries = np.asarray(queries, dtype=np.float32)
    keys = np.asarray(keys, dtype=np.float32)
    values = np.asarray(values, dtype=np.float32)
    valid_lens = np.asarray(valid_lens, dtype=np.int32)

    assign, m_list = _plan(valid_lens)

    key = tuple(m_list)
    nc = _program_cache.get(key)
    if nc is None:
        nc = _build_program(m_list)
        _program_cache[key] = nc

    q16 = np.ascontiguousarray(queries.transpose(0, 2, 1)).astype(np.float16)
    k16 = np.ascontiguousarray(keys.transpose(0, 2, 1)).astype(np.float16)
    v16 = values.astype(np.float16)
    kk = np.arange(L, dtype=np.int64)

    in_maps = []
    for i in range(NCORES):
        heads = assign[i]
        qk = np.zeros((SLOTS, 128, QKW), dtype=np.float16)
        vp = np.zeros((SLOTS, 128, VPW), dtype=np.float16)
        for j in range(SLOTS):
            h = heads[j]
            m = m_list[j]
            qk[j, 0:64, 0:L] = q16[h]
            qk[j, 64:128, 0:L] = q16[h]
            for c in range(m):
                half = (c % 2) * 64
                g = c // 2
                qk[j, half : half + 64, L + g * CHUNK : L + (g + 1) * CHUNK] = k16[
                    h, :, c * CHUNK : (c + 1) * CHUNK
                ]
            v3 = vp[j, :, 0 : m * 65].reshape(128, m, 65)
            v3[:, :, 0:D] = v16[h, 0 : m * CHUNK].reshape(m, CHUNK, D).transpose(
                1, 0, 2
            )
            v3[:, :, D] = 1.0
        mask = np.where(
            kk[None, :] < valid_lens[heads][:, None], 0.0, MASK_VALUE
        ).astype(np.float32)  # [SLOTS, L]
        # mb[p, j*NCH+c] = mask for key index c*128+p of slot j.
        mb = np.transpose(mask.reshape(SLOTS, NCH, CHUNK), (2, 0, 1)).reshape(
            CHUNK, SLOTS * NCH
        )
        in_maps.append(
            {
                "qk": qk,
                "vp": vp,
                "mb": np.ascontiguousarray(mb),
            }
        )

    res = run_bass_kernel_spmd(nc, in_maps, list(range(NCORES)), trace=trace)

    out = np.empty((BH, L, D), dtype=np.float32)
    for i in range(NCORES):
        ot = res.results[i]["ot"].astype(np.float32)  # [SLOTS, 65, 1024]
        o = ot[:, 0:D, :] / ot[:, D : D + 1, :]  # normalize
        out[assign[i]] = o.transpose(0, 2, 1)

    # valid_len == 0: reference softmaxes an all-masked row -> uniform weights.
    for h in np.nonzero(valid_lens == 0)[0]:
        out[h] = values[h].mean(axis=0, keepdims=True)

    return out, res


def kernel(queries, keys, values, valid_lens):
    out, _ = _run(queries, keys, values, valid_lens)
    return out


# revision 27
# speedup vs baseline: 1.0121x; 1.0121x over previous
"""Bass/Tile kernel for masked dot-product attention on 8 Trainium2 cores.

Problem: queries/keys/values [128, 1024, 64] fp32, valid_lens [128] int32.
  out[b] = softmax(mask(Q K^T / 8, valid_lens[b])) @ V

Strategy (v2):
  * Shard the 128 batch*heads across 8 cores, 16 head-slots per core.
    Heads are sorted by valid_len (descending) and dealt in consecutive
    groups of 8, one head per core per slot -> one SPMD program whose
    per-slot key-chunk count m = ceil(max valid_len in group / 128).
  * All layout work happens on the HOST: Q^T / K^T fp16 panels and
    [V_chunk | ones] fp16 blocks are built with numpy, so the device does
    zero transposes, zero casts of inputs, zero memsets per head.
  * Device pipeline per key chunk c (the ScalarE exp is the pacer):
      S^T = K_c Q^T            2 matmuls  (PE, K=64, N=512, fp16)
      P^T = exp(S^T/8 + mask)  1 activation [128,1024] (ACT, bias = mask col)
      [O^T ; denom] += [V_c|1]^T P^T   2 matmuls (PE, M=65, N=512)
    The masked key positions get bias -1e6 -> exp underflows to exactly 0,
    so fully-masked chunks beyond valid_len are simply skipped and no max
    subtraction is needed (scores are bounded, fp32 reference-exact).
  * The denominator rides along as a 65th stationary column of ones, so
    one PSUM accumulator holds [O^T ; sum_k P^T].  The device just casts
    it to fp16 and ships it; normalization (num/den) and the final
    transpose back to [q, d] happen on the host in numpy.
  * Heads with valid_len == 0 (reference: uniform attention) are fixed up
    on the host with the exact reference semantics (mean of V).
"""

import math
from contextlib import ExitStack

import numpy as np

import concourse.bass as bass  # noqa: F401  (engine namespaces live on the nc)
import concourse.mybir as mybir
import concourse.tile as tile
from concourse import bacc
from concourse.bass_utils import run_bass_kernel_spmd

BH, L, D = 128, 1024, 64
NCORES = 8
SLOTS = BH // NCORES  # 16
CHUNK = 128
NCH = L // CHUNK  # 8
MASK_VALUE = -1000000.0
F32 = mybir.dt.float32
MM_DT = mybir.dt.float16  # 1 cyc/row on PE, ~2^-11 operand quantization

NPAIR = NCH // 2  # key chunks are processed in row-tiled pairs
QKW = L + NPAIR * CHUNK  # qk panel max width: Q^T (1024) + K^T pair blocks
VPW = NCH * (D + 1)  # vp panel max width: [V_c | ones] blocks of 65

_program_cache: dict = {}


def _build_program(m_list):
    nc = bacc.Bacc("TRN2", target_bir_lowering=False, debug=False)
    qk_d = nc.dram_tensor("qk", [SLOTS, 128, QKW], MM_DT, kind="ExternalInput").ap()
    vp_d = nc.dram_tensor("vp", [SLOTS, 128, VPW], MM_DT, kind="ExternalInput").ap()
    mb_d = nc.dram_tensor("mb", [CHUNK, SLOTS * NCH], F32, kind="ExternalInput").ap()
    ot_d = nc.dram_tensor("ot", [SLOTS, D + 1, L], MM_DT, kind="ExternalOutput").ap()

    Exp = mybir.ActivationFunctionType.Exp

    with tile.TileContext(nc) as tc, ExitStack() as ctx:
        const = ctx.enter_context(tc.tile_pool(name="const", bufs=1))
        mb = const.tile([CHUNK, SLOTS * NCH], F32)
        nc.sync.dma_start(mb[:], mb_d[:])
        ones = const.tile([128, 1], F32)
        nc.vector.memset(ones[:], 1.0)
        actwarm = const.tile([128, 1], F32, tag="actwarm")
        nc.scalar.activation(actwarm[:], ones[:], Exp, bias=0.0, scale=1.0)

        qk_p = ctx.enter_context(tc.tile_pool(name="qk", bufs=3))
        vp_p = ctx.enter_context(tc.tile_pool(name="vp", bufs=3))
        pts_p = ctx.enter_context(tc.tile_pool(name="pts", bufs=5))
        osb_p = ctx.enter_context(tc.tile_pool(name="osb", bufs=3))

        # PSUM: 8 banks. s: S^T tiles (2 x 2 banks); ot: [O^T;den]
        # accumulators, one bank per q-half, double-buffered across heads.
        s_ps = ctx.enter_context(tc.tile_pool(name="s", bufs=3, space="PSUM"))
        o_ps = ctx.enter_context(tc.tile_pool(name="ot", bufs=2, space="PSUM"))

        def load_head(j):
            g = (m_list[j] + 1) // 2  # pair blocks
            qk = qk_p.tile([128, QKW], MM_DT, tag="qk", name=f"qk{j}")
            nc.sync.dma_start(
                qk[:, 0 : L + g * CHUNK], qk_d[j, :, 0 : L + g * CHUNK]
            )
            vp = vp_p.tile([128, VPW], MM_DT, tag="vp", name=f"vp{j}")
            nc.gpsimd.dma_start(vp[:, 0 : m_list[j] * 65], vp_d[j, :, 0 : m_list[j] * 65])
            return qk, vp

        heads = {j: None for j in range(SLOTS)}
        for j in range(min(3, SLOTS)):
            heads[j] = load_head(j)

        # Dense matmul burst to flip the PE HAM clock-gate to full rate
        # (needs ~3.4us of contiguous PE activity; at the cold 1.2 GHz rate
        # 6 x N=512 matmuls ~= 3.8us) while the first panel DMAs are in
        # flight.  Too short a burst leaves the whole kernel at K=4/8.
        warm = const.tile([128, 512], MM_DT, tag="warm")
        nc.vector.memset(warm[:], 0.5)
        wps = o_ps.tile([128, 512], F32, tag="ot")  # noqa
        for i in range(6):
            nc.tensor.matmul(wps[:], warm[:, 0:128], warm[:], start=True, stop=True)

        # Key chunks are processed in pairs: the even chunk's K^T lives in
        # SBUF partitions 0:64, the odd chunk's in 64:128 (Q^T is duplicated
        # in both halves).  The paired S^T matmuls go to different PE row
        # groups (tile_position (0,0) / (64,0)) and run concurrently.
        pairs = []
        for j in range(SLOTS):
            for g in range((m_list[j] + 1) // 2):
                cs = [2 * g] + ([2 * g + 1] if 2 * g + 1 < m_list[j] else [])
                pairs.append((j, g, cs))
        state: dict = {}  # (j, c) -> pts tile
        otiles: dict = {}  # j -> [ot_h0, ot_h1]

        def emit_pv(j, c):
            qk, vp = heads[j]
            m = m_list[j]
            if c == 0:
                otiles[j] = [
                    o_ps.tile([D + 1, 512], F32, tag="ot", name=f"ot{j}_{h}")
                    for h in range(2)
                ]
            pts = state.pop((j, c))
            for h in range(2):
                nc.tensor.matmul(
                    otiles[j][h][:],
                    vp[:, c * 65 : (c + 1) * 65],
                    pts[:, h * 512 : (h + 1) * 512],
                    start=(c == 0),
                    stop=(c == m - 1),
                )
            if c == m - 1:
                # [O^T ; denom] -> fp16 -> DRAM; host divides + transposes.
                osb = osb_p.tile([D + 1, L], MM_DT, tag="osb", name=f"osb{j}")
                for h in range(2):
                    nc.vector.tensor_copy(
                        osb[:, h * 512 : (h + 1) * 512], otiles[j][h][:]
                    )
                    nc.sync.dma_start(
                        ot_d[j, :, h * 512 : (h + 1) * 512],
                        osb[:, h * 512 : (h + 1) * 512],
                    )
                del otiles[j]

        prev_pair = None
        cur_head = -1
        for j, g, cs in pairs:
            if j != cur_head:
                cur_head = j
                if j + 3 < SLOTS:
                    heads[j + 3] = load_head(j + 3)
            qk, _ = heads[j]
            stiles = {
                c: s_ps.tile([128, L], F32, tag="s", name=f"s{j}_{c}") for c in cs
            }
            for h in range(2):
                for c in cs:
                    half = (c % 2) * 64
                    nc.tensor.matmul(
                        stiles[c][:, h * 512 : (h + 1) * 512],
                        qk[half : half + 64, L + g * CHUNK : L + (g + 1) * CHUNK],
                        qk[half : half + 64, h * 512 : (h + 1) * 512],
                        start=True,
                        stop=True,
                    )
            for c in cs:
                pts = pts_p.tile([128, L], MM_DT, tag="pts", name=f"pts{j}_{c}")
                col = j * NCH + c
                nc.scalar.activation(
                    pts[:], stiles[c][:], Exp, bias=mb[:, col : col + 1], scale=0.125
                )
                state[(j, c)] = pts
            if prev_pair is not None:
                for pj, pc in prev_pair:
                    emit_pv(pj, pc)
            prev_pair = [(j, c) for c in cs]
        for pj, pc in prev_pair:
            emit_pv(pj, pc)

    nc.compile()
    return nc


def _plan(valid_lens):
    """Sort heads by valid_len desc, deal consecutive groups of 8 across cores.

    Returns (assign [NCORES, SLOTS] head indices, m_list [SLOTS] chunk counts).
    """
    order = np.argsort(-valid_lens, kind="stable")
    assign = order.reshape(SLOTS, NCORES).T  # [core, slot]
    m_list = []
    for j in range(SLOTS):
        vmax = int(valid_lens[assign[:, j]].max())
        m_list.append(min(NCH, max(1, math.ceil(vmax / CHUNK))))
    return assign, m_list


def _run(queries, keys, values, valid_lens, trace=False):
    queries = np.asarray(queries, dtype=np.float32)
    keys = np.asarray(keys, dtype=np.float32)
    values = np.asarray(values, dtype=np.float32)
    valid_lens = np.asarray(valid_lens, dtype=np.int32)

    assign, m_list = _plan(valid_lens)

    key = tuple(m_list)
    nc = _program_cache.get(key)
    if nc is None:
        nc = _build_program(m_list)
        _program_cache[key] = nc

    q16 = np.ascontiguousarray(queries.transpose(0, 2, 1)).astype(np.float16)
    k16 = np.ascontiguousarray(keys.transpose(0, 2, 1)).astype(np.float16)
    v16 = values.astype(np.float16)
    kk = np.arange(L, dtype=np.int64)

    in_maps = []
    for i in range(NCORES):
        heads = assign[i]
        qk = np.zeros((SLOTS, 128, QKW), dtype=np.float16)
        vp = np.zeros((SLOTS, 128, VPW), dtype=np.float16)
        for j in range(SLOTS):
            h = heads[j]
            m = m_list[j]
            qk[j, 0:64, 0:L] = q16[h]
            qk[j, 64:128, 0:L] = q16[h]
            for c in range(m):
                half = (c % 2) * 64
                g = c // 2
                qk[j, half : half + 64, L + g * CHUNK : L + (g + 1) * CHUNK] = k16[
                    h, :, c * CHUNK : (c + 1) * CHUNK
                ]
            v3 = vp[j, :, 0 : m * 65].reshape(128, m, 65)
            v3[:, :, 0:D] = v16[h, 0 : m * CHUNK].reshape(m, CHUNK, D).transpose(
                1, 0, 2
            )
            v3[:, :, D] = 1.0
        mask = np.where(
            kk[None, :] < valid_lens[heads][:, None], 0.0, MASK_VALUE
        ).astype(np.float32)  # [SLOTS, L]
        # mb[p, j*NCH+c] = mask for key index c*128+p of slot j.
        mb = np.transpose(mask.reshape(SLOTS, NCH, CHUNK), (2, 0, 1)).reshape(
            CHUNK, SLOTS * NCH
        )
        in_maps.append(
            {
                "qk": qk,
                "vp": vp,
                "mb": np.ascontiguousarray(mb),
            }
        )

    res = run_bass_kernel_spmd(nc, in_maps, list(range(NCORES)), trace=trace)

    out = np.empty((BH, L, D), dtype=np.float32)
    for i in range(NCORES):
        ot = res.results[i]["ot"].astype(np.float32)  # [SLOTS, 65, 1024]
        o = ot[:, 0:D, :] / ot[:, D : D + 1, :]  # normalize
        out[assign[i]] = o.transpose(0, 2, 1)

    # valid_len == 0: reference softmaxes an all-masked row -> uniform weights.
    for h in np.nonzero(valid_lens == 0)[0]:
        out[h] = values[h].mean(axis=0, keepdims=True)

    return out, res


def kernel(queries, keys, values, valid_lens):
    out, _ = _run(queries, keys, values, valid_lens)
    return out


# revision 28
# speedup vs baseline: 1.0369x; 1.0245x over previous
"""Bass/Tile kernel for masked dot-product attention on 8 Trainium2 cores.

Problem: queries/keys/values [128, 1024, 64] fp32, valid_lens [128] int32.
  out[b] = softmax(mask(Q K^T / 8, valid_lens[b])) @ V

Strategy (v2):
  * Shard the 128 batch*heads across 8 cores, 16 head-slots per core.
    Heads are sorted by valid_len (descending) and dealt in consecutive
    groups of 8, one head per core per slot -> one SPMD program whose
    per-slot key-chunk count m = ceil(max valid_len in group / 128).
  * All layout work happens on the HOST: Q^T / K^T fp16 panels and
    [V_chunk | ones] fp16 blocks are built with numpy, so the device does
    zero transposes, zero casts of inputs, zero memsets per head.
  * Device pipeline per key chunk c (the ScalarE exp is the pacer):
      S^T = K_c Q^T            2 matmuls  (PE, K=64, N=512, fp16)
      P^T = exp(S^T/8 + mask)  1 activation [128,1024] (ACT, bias = mask col)
      [O^T ; denom] += [V_c|1]^T P^T   2 matmuls (PE, M=65, N=512)
    The masked key positions get bias -1e6 -> exp underflows to exactly 0,
    so fully-masked chunks beyond valid_len are simply skipped and no max
    subtraction is needed (scores are bounded, fp32 reference-exact).
  * The denominator rides along as a 65th stationary column of ones, so
    one PSUM accumulator holds [O^T ; sum_k P^T].  The device just casts
    it to fp16 and ships it; normalization (num/den) and the final
    transpose back to [q, d] happen on the host in numpy.
  * Heads with valid_len == 0 (reference: uniform attention) are fixed up
    on the host with the exact reference semantics (mean of V).
"""

import math
from contextlib import ExitStack

import numpy as np

import concourse.bass as bass  # noqa: F401  (engine namespaces live on the nc)
import concourse.mybir as mybir
import concourse.tile as tile
from concourse import bacc
from concourse.bass_utils import run_bass_kernel_spmd

BH, L, D = 128, 1024, 64
NCORES = 8
SLOTS = BH // NCORES  # 16
CHUNK = 128
NCH = L // CHUNK  # 8
MASK_VALUE = -1000000.0
F32 = mybir.dt.float32
MM_DT = mybir.dt.float16  # 1 cyc/row on PE, ~2^-11 operand quantization

NPAIR = NCH // 2  # key chunks are processed in row-tiled pairs
QKW = L + NPAIR * CHUNK  # qk panel max width: Q^T (1024) + K^T pair blocks
VPW = NCH * (D + 1)  # vp panel max width: [V_c | ones] blocks of 65

# Schraudolph fast-exp: exp(x) ~= bitcast_f32(int32(x*2^23/ln2 + B)).
# A0 folds the 1/sqrt(d)=0.125 score scale; B centers the mantissa sawtooth
# (max rel err ~3%; end-to-end output err ~3e-3, well under the 2e-2 gate).
SCHRAUDOLPH_A0 = float(2.0**23 / math.log(2.0) * 0.125)
SCHRAUDOLPH_B = float(127 * 2**23 - 366393)
MASKED_BIAS = 1.0e8  # masked keys: int32 lands in f32-denormal land -> fp16 0


def dve_cs(m):
    """Key chunks whose exp runs on the Vector engine instead of ScalarE
    (the pipeline pacer).  Only long heads donate chunks: the DVE path is
    ~2.4x the ACT cost and the per-head output casts also live on the DVE."""
    if m >= 8:
        return (1, 3)
    if m >= 5:
        return (1,)
    return ()


_program_cache: dict = {}


def _build_program(m_list):
    nc = bacc.Bacc("TRN2", target_bir_lowering=False, debug=False)
    qk_d = nc.dram_tensor("qk", [SLOTS, 128, QKW], MM_DT, kind="ExternalInput").ap()
    vp_d = nc.dram_tensor("vp", [SLOTS, 128, VPW], MM_DT, kind="ExternalInput").ap()
    mb_d = nc.dram_tensor("mb", [CHUNK, SLOTS * NCH], F32, kind="ExternalInput").ap()
    mb2_d = nc.dram_tensor("mb2", [CHUNK, SLOTS * NCH], F32, kind="ExternalInput").ap()
    ot_d = nc.dram_tensor("ot", [SLOTS, D + 1, L], MM_DT, kind="ExternalOutput").ap()

    Exp = mybir.ActivationFunctionType.Exp

    with tile.TileContext(nc) as tc, ExitStack() as ctx:
        const = ctx.enter_context(tc.tile_pool(name="const", bufs=1))
        mb = const.tile([CHUNK, SLOTS * NCH], F32)
        nc.sync.dma_start(mb[:], mb_d[:])
        mb2 = const.tile([CHUNK, SLOTS * NCH], F32)
        nc.sync.dma_start(mb2[:], mb2_d[:])
        ones = const.tile([128, 1], F32)
        nc.vector.memset(ones[:], 1.0)
        actwarm = const.tile([128, 1], F32, tag="actwarm")
        nc.scalar.activation(actwarm[:], ones[:], Exp, bias=0.0, scale=1.0)

        qk_p = ctx.enter_context(tc.tile_pool(name="qk", bufs=3))
        vp_p = ctx.enter_context(tc.tile_pool(name="vp", bufs=3))
        pts_p = ctx.enter_context(tc.tile_pool(name="pts", bufs=5))
        i32_p = ctx.enter_context(tc.tile_pool(name="i32", bufs=2))
        osb_p = ctx.enter_context(tc.tile_pool(name="osb", bufs=3))

        # PSUM: 8 banks. s: S^T tiles (2 x 2 banks); ot: [O^T;den]
        # accumulators, one bank per q-half, double-buffered across heads.
        s_ps = ctx.enter_context(tc.tile_pool(name="s", bufs=3, space="PSUM"))
        o_ps = ctx.enter_context(tc.tile_pool(name="ot", bufs=2, space="PSUM"))

        def load_head(j):
            g = (m_list[j] + 1) // 2  # pair blocks
            qk = qk_p.tile([128, QKW], MM_DT, tag="qk", name=f"qk{j}")
            nc.sync.dma_start(
                qk[:, 0 : L + g * CHUNK], qk_d[j, :, 0 : L + g * CHUNK]
            )
            vp = vp_p.tile([128, VPW], MM_DT, tag="vp", name=f"vp{j}")
            nc.gpsimd.dma_start(vp[:, 0 : m_list[j] * 65], vp_d[j, :, 0 : m_list[j] * 65])
            return qk, vp

        heads = {j: None for j in range(SLOTS)}
        for j in range(min(3, SLOTS)):
            heads[j] = load_head(j)

        # Dense matmul burst to flip the PE HAM clock-gate to full rate
        # (needs ~3.4us of contiguous PE activity; at the cold 1.2 GHz rate
        # 6 x N=512 matmuls ~= 3.8us) while the first panel DMAs are in
        # flight.  Too short a burst leaves the whole kernel at K=4/8.
        warm = const.tile([128, 512], MM_DT, tag="warm")
        nc.vector.memset(warm[:], 0.5)
        wps = o_ps.tile([128, 512], F32, tag="ot")  # noqa
        for i in range(11):
            nc.tensor.matmul(wps[:], warm[:, 0:128], warm[:], start=True, stop=True)

        # Key chunks are processed in pairs: the even chunk's K^T lives in
        # SBUF partitions 0:64, the odd chunk's in 64:128 (Q^T is duplicated
        # in both halves).  The paired S^T matmuls go to different PE row
        # groups (tile_position (0,0) / (64,0)) and run concurrently.
        pairs = []
        for j in range(SLOTS):
            for g in range((m_list[j] + 1) // 2):
                cs = [2 * g] + ([2 * g + 1] if 2 * g + 1 < m_list[j] else [])
                pairs.append((j, g, cs))
        state: dict = {}  # (j, c) -> pts tile
        otiles: dict = {}  # j -> [ot_h0, ot_h1]

        def emit_pv(j, c):
            qk, vp = heads[j]
            m = m_list[j]
            if c == 0:
                otiles[j] = [
                    o_ps.tile([D + 1, 512], F32, tag="ot", name=f"ot{j}_{h}")
                    for h in range(2)
                ]
            pts = state.pop((j, c))
            for h in range(2):
                nc.tensor.matmul(
                    otiles[j][h][:],
                    vp[:, c * 65 : (c + 1) * 65],
                    pts[:, h * 512 : (h + 1) * 512],
                    start=(c == 0),
                    stop=(c == m - 1),
                )
            if c == m - 1:
                # [O^T ; denom] -> fp16 -> DRAM; host divides + transposes.
                osb = osb_p.tile([D + 1, L], MM_DT, tag="osb", name=f"osb{j}")
                for h in range(2):
                    nc.vector.tensor_copy(
                        osb[:, h * 512 : (h + 1) * 512], otiles[j][h][:]
                    )
                    nc.sync.dma_start(
                        ot_d[j, :, h * 512 : (h + 1) * 512],
                        osb[:, h * 512 : (h + 1) * 512],
                    )
                del otiles[j]

        prev_pair = None
        cur_head = -1
        for j, g, cs in pairs:
            if j != cur_head:
                cur_head = j
                if j + 3 < SLOTS:
                    heads[j + 3] = load_head(j + 3)
            qk, _ = heads[j]
            stiles = {
                c: s_ps.tile([128, L], F32, tag="s", name=f"s{j}_{c}") for c in cs
            }
            for h in range(2):
                for c in cs:
                    half = (c % 2) * 64
                    nc.tensor.matmul(
                        stiles[c][:, h * 512 : (h + 1) * 512],
                        qk[half : half + 64, L + g * CHUNK : L + (g + 1) * CHUNK],
                        qk[half : half + 64, h * 512 : (h + 1) * 512],
                        start=True,
                        stop=True,
                    )
            for c in cs:
                pts = pts_p.tile([128, L], MM_DT, tag="pts", name=f"pts{j}_{c}")
                col = j * NCH + c
                if c in dve_cs(m_list[j]):
                    # Schraudolph exp on the Vector engine: bitcast(int32(
                    # s*A0 + bias)) ~= exp(s/8 + mask); the masked bias lands
                    # the int in f32-denormal land -> 0 after the fp16 cast.
                    i32 = i32_p.tile(
                        [128, L], mybir.dt.int32, tag="i32", name=f"i32_{j}_{c}"
                    )
                    nc.vector.tensor_scalar(
                        i32[:],
                        stiles[c][:],
                        SCHRAUDOLPH_A0,
                        mb2[:, col : col + 1],
                        mybir.AluOpType.mult,
                        mybir.AluOpType.add,
                    )
                    nc.vector.tensor_copy(pts[:], i32[:].bitcast(F32))
                else:
                    nc.scalar.activation(
                        pts[:],
                        stiles[c][:],
                        Exp,
                        bias=mb[:, col : col + 1],
                        scale=0.125,
                    )
                state[(j, c)] = pts
            if prev_pair is not None:
                for pj, pc in prev_pair:
                    emit_pv(pj, pc)
            prev_pair = [(j, c) for c in cs]
        for pj, pc in prev_pair:
            emit_pv(pj, pc)

    nc.compile()
    return nc


def _plan(valid_lens):
    """Sort heads by valid_len desc, deal consecutive groups of 8 across cores.

    Returns (assign [NCORES, SLOTS] head indices, m_list [SLOTS] chunk counts).
    """
    order = np.argsort(-valid_lens, kind="stable")
    assign = order.reshape(SLOTS, NCORES).T  # [core, slot]
    m_list = []
    for j in range(SLOTS):
        vmax = int(valid_lens[assign[:, j]].max())
        m_list.append(min(NCH, max(1, math.ceil(vmax / CHUNK))))
    return assign, m_list


def _run(queries, keys, values, valid_lens, trace=False):
    queries = np.asarray(queries, dtype=np.float32)
    keys = np.asarray(keys, dtype=np.float32)
    values = np.asarray(values, dtype=np.float32)
    valid_lens = np.asarray(valid_lens, dtype=np.int32)

    assign, m_list = _plan(valid_lens)

    key = tuple(m_list)
    nc = _program_cache.get(key)
    if nc is None:
        nc = _build_program(m_list)
        _program_cache[key] = nc

    q16 = np.ascontiguousarray(queries.transpose(0, 2, 1)).astype(np.float16)
    k16 = np.ascontiguousarray(keys.transpose(0, 2, 1)).astype(np.float16)
    v16 = values.astype(np.float16)
    kk = np.arange(L, dtype=np.int64)

    in_maps = []
    for i in range(NCORES):
        heads = assign[i]
        qk = np.zeros((SLOTS, 128, QKW), dtype=np.float16)
        vp = np.zeros((SLOTS, 128, VPW), dtype=np.float16)
        for j in range(SLOTS):
            h = heads[j]
            m = m_list[j]
            qk[j, 0:64, 0:L] = q16[h]
            qk[j, 64:128, 0:L] = q16[h]
            for c in range(m):
                half = (c % 2) * 64
                g = c // 2
                qk[j, half : half + 64, L + g * CHUNK : L + (g + 1) * CHUNK] = k16[
                    h, :, c * CHUNK : (c + 1) * CHUNK
                ]
            v3 = vp[j, :, 0 : m * 65].reshape(128, m, 65)
            v3[:, :, 0:D] = v16[h, 0 : m * CHUNK].reshape(m, CHUNK, D).transpose(
                1, 0, 2
            )
            v3[:, :, D] = 1.0
        valid = kk[None, :] < valid_lens[heads][:, None]  # [SLOTS, L]
        mask = np.where(valid, 0.0, MASK_VALUE).astype(np.float32)
        mask2 = np.where(valid, SCHRAUDOLPH_B, MASKED_BIAS).astype(np.float32)

        # panel[p, j*NCH+c] = value for key index c*128+p of slot j.
        def panel(a):
            return np.ascontiguousarray(
                np.transpose(a.reshape(SLOTS, NCH, CHUNK), (2, 0, 1)).reshape(
                    CHUNK, SLOTS * NCH
                )
            )

        in_maps.append(
            {"qk": qk, "vp": vp, "mb": panel(mask), "mb2": panel(mask2)}
        )

    res = run_bass_kernel_spmd(nc, in_maps, list(range(NCORES)), trace=trace)

    out = np.empty((BH, L, D), dtype=np.float32)
    for i in range(NCORES):
        ot = res.results[i]["ot"].astype(np.float32)  # [SLOTS, 65, 1024]
        o = ot[:, 0:D, :] / ot[:, D : D + 1, :]  # normalize
        out[assign[i]] = o.transpose(0, 2, 1)

    # valid_len == 0: reference softmaxes an all-masked row -> uniform weights.
    for h in np.nonzero(valid_lens == 0)[0]:
        out[h] = values[h].mean(axis=0, keepdims=True)

    return out, res


def kernel(queries, keys, values, valid_lens):
    out, _ = _run(queries, keys, values, valid_lens)
    return out
